# revision 1
# baseline (speedup 1.0000x reference)
"""CapsuleLayer (dynamic routing) Trainium2 kernel.

Math: reference routing updates B_logits += exp(-d2) where
d2 = |prior - out|^2 per (b, c, r). For these magnitudes d2 is ~chi^2
distributed around 128, so exp(-d2) underflows reference f32 for all but a
sparse set of triples (d2 < ~19 is the f32-visible cutoff). Device computes
  s_sum[b,c,o] = sum_r priors[b,c,r,o]        (exact f32 matmul)
  q[b,c,r]     = |priors[b,c,r,:]|_1          (bf16 block-diag matmul + abs
                                               reduce; threshold only —
                                               Cauchy-Schwarz bounds L2)
R-sharded over 8 cores (zero input replication). Host gathers, finds the
sparse set {q < THETA}, recomputes those priors exactly in f64, and runs the
exact 3-iteration routing with sparse softmax corrections.
"""

import sys
import functools

sys.path.insert(0, "/opt/trn_rl_repo")

import numpy as np
import ml_dtypes

B, C, R, I, O = 128, 10, 4608, 8, 16
NCORES = 8
RL = R // NCORES            # 576 route nodes per core
RCHUNK = RL // 16           # 36 chunks of 16 r (=128 contraction rows)
ROUTE_ITERATIONS = 3
SIGMA = 1.0
THETA = 20.0                # |p|_1 threshold: d2<20 => |p|_1 < 18.6 (C-S)

LAST_RESULTS = None         # BassKernelResults of the most recent run (for test)


def _build_nc(reps=1, parts="full"):
    import concourse.bass as bass
    import concourse.mybir as mybir
    from concourse.tile import TileContext
    from concourse.masks import make_identity

    f32 = mybir.dt.float32
    bf16 = mybir.dt.bfloat16
    CO = C * O              # 160
    NB = 256                # block-diag matmul free size = 16 r * 16 o

    nc = bass.Bass(trn_type="TRN2")
    xs = nc.dram_tensor("xs", [B, RL * I], f32, kind="ExternalInput")
    ws = nc.dram_tensor("ws", [RCHUNK, 128, CO], f32, kind="ExternalInput")
    mk = nc.dram_tensor("mk", [128, C * NB], f32, kind="ExternalInput")
    s_out = nc.dram_tensor("s_out", [B, CO], f32, kind="ExternalOutput")
    q_out = nc.dram_tensor("q_out", [RCHUNK, B, C * 16], f32, kind="ExternalOutput")

    GRP = 6                 # rc chunks per W preload DMA group

    with TileContext(nc) as tc:
        with (
            tc.tile_pool(name="const", bufs=1) as constp,
            tc.tile_pool(name="wblk", bufs=2) as wblkp,
            tc.tile_pool(name="sq", bufs=RCHUNK) as sqp,
            tc.tile_pool(name="qsb", bufs=RCHUNK // 2) as qsbp,
            tc.tile_pool(name="ps_s", bufs=1, space="PSUM") as ps_s,
            tc.tile_pool(name="ps_t", bufs=2, space="PSUM") as ps_t,
            tc.tile_pool(name="ps_p", bufs=2, space="PSUM") as ps_p,
            tc.tile_pool(name="ps_q", bufs=1, space="PSUM") as ps_q,
        ):
            ident = constp.tile([128, 128], f32)
            make_identity(nc, ident[:])
            mask = constp.tile([128, C * NB], f32)
            nc.sync.dma_start(mask[:], mk[:])
            xs_sb = constp.tile([B, RL * I], f32)
            nc.sync.dma_start(xs_sb[:], xs[:])

            # Preload all W in GRP-chunk group DMAs (independent tiles).
            ws_g = []
            for g in range(RCHUNK // GRP):
                wsg = constp.tile([128, GRP * CO], f32, tag=f"wsg{g}")
                nc.sync.dma_start(
                    wsg[:].rearrange("p (rc co) -> p rc co", rc=GRP),
                    ws[g * GRP:(g + 1) * GRP].rearrange("rc p co -> p rc co"),
                )
                ws_g.append(wsg)

            # PE wait-absorbers: any instruction carries at most one
            # sync-wait, so observe the identity (Pool) and xs (DMA)
            # semaphores on dummy bf16 ldweights (no PSUM write; every real
            # matmul re-embeds its own weight load).
            nc.tensor.ldweights(ident[:, 0:64].bitcast(bf16))
            nc.tensor.ldweights(xs_sb[:, 0:64].bitcast(bf16))

            s_psum = ps_s.tile([128, CO], f32)

            for rep in range(reps):
                # ---- Phase 1: transpose all x chunks, one bulk bf16 cast ----
                xt = constp.tile([128, RL * I], f32, tag="xt")
                for rcp in range(RCHUNK // 2):
                    tp = ps_t.tile([128, 256], f32, tag="tp")
                    for k in range(2):
                        rc = rcp * 2 + k
                        nc.tensor.transpose(
                            tp[:, k * 128:(k + 1) * 128],
                            xs_sb[:, rc * 128:(rc + 1) * 128], ident[:])
                    nc.vector.tensor_copy(
                        xt[:, rcp * 256:(rcp + 1) * 256], tp[:])
                xt16 = constp.tile([128, RL * I], bf16, tag="xt16")
                nc.scalar.copy(xt16[:], xt[:])
                # Let the PE observe the bulk cast once (ACT) and DVE (xt).
                nc.tensor.ldweights(xt16[:, 0:128])

                # ---- Phase 2: s-matmuls + block-diag q pipeline ----
                for rc2 in range(RCHUNK // 2):
                    qsb = qsbp.tile([B, 2 * C * 16], f32)
                    # pair-merged block-diagonal construction (one DVE op)
                    rc0 = rc2 * 2
                    g, gi = divmod(rc0, GRP)
                    wsg = ws_g[g]
                    if gi == 0:
                        # Absorb this W group's DMA semaphore.
                        nc.tensor.ldweights(wsg[:, 0:64].bitcast(bf16))
                    wblk = wblkp.tile([128, 2 * C * NB], bf16)
                    w_b = (
                        wsg[:, gi * CO:(gi + 2) * CO]
                        .rearrange("p (rc c o) -> p rc c o", rc=2, c=C)
                        .unsqueeze(3)
                        .broadcast_to((128, 2, C, 16, O))
                    )
                    m_b = (
                        mask[:].rearrange("p (c r o) -> p c r o", c=C, r=16)
                        .unsqueeze(1)
                        .broadcast_to((128, 2, C, 16, O))
                    )
                    nc.vector.tensor_tensor(
                        wblk[:].rearrange(
                            "p (rc c r o) -> p rc c r o", rc=2, c=C, r=16),
                        w_b, m_b, mybir.AluOpType.mult,
                    )
                    for k in range(2):
                        rc = rc0 + k
                        xt_sl = xt[:, rc * 128:(rc + 1) * 128]
                        nc.tensor.matmul(
                            s_psum[:], xt_sl, wsg[:, (gi + k) * CO:(gi + k + 1) * CO],
                            start=(rc == 0), stop=(rc == RCHUNK - 1),
                            skip_group_check=True,
                        )
                        xt16_sl = xt16[:, rc * 128:(rc + 1) * 128]
                        wb0 = k * C * NB
                        qb0 = k * C * 16
                        for half in range(2):
                            # 4 capsules per 2-bank PSUM tile, one L1 reduce
                            pp = ps_p.tile([128, 4 * NB], f32, tag="pp")
                            for j in range(2):
                                c0 = half * 4 + j * 2
                                nc.tensor.matmul(
                                    pp[:, j * 2 * NB:(j + 1) * 2 * NB],
                                    xt16_sl,
                                    wblk[:, wb0 + c0 * NB:wb0 + (c0 + 2) * NB],
                                    start=True, stop=True,
                                    skip_group_check=True,
                                )
                            if parts == "pmm":
                                continue
                            nc.vector.tensor_reduce(
                                qsb[:, qb0 + half * 64:qb0 + half * 64 + 64],
                                pp[:].rearrange(
                                    "p (cc r o) -> p cc r o", cc=4, o=O),
                                mybir.AxisListType.X,
                                mybir.AluOpType.add,
                                apply_absolute_value=True,
                            )
                        # leftover capsules 8,9
                        pps = ps_q.tile([128, 2 * NB], f32, tag="pps")
                        nc.tensor.matmul(
                            pps[:], xt16_sl,
                            wblk[:, wb0 + 8 * NB:wb0 + 10 * NB],
                            start=True, stop=True, skip_group_check=True,
                        )
                        if parts != "pmm":
                            nc.vector.tensor_reduce(
                                qsb[:, qb0 + 128:qb0 + 160],
                                pps[:].rearrange(
                                    "p (cc r o) -> p cc r o", cc=2, o=O),
                                mybir.AxisListType.X,
                                mybir.AluOpType.add,
                                apply_absolute_value=True,
                            )
                    if parts != "pmm":
                        nc.sync.dma_start(
                            q_out[rc0:rc0 + 2].rearrange("rc b f -> b rc f"),
                            qsb[:].rearrange("b (rc f) -> b rc f", rc=2),
                        )

            s_sb = constp.tile([B, CO], f32)
            nc.vector.tensor_copy(s_sb[:], s_psum[:])
            nc.sync.dma_start(s_out[:], s_sb[:])

    _split_multi_waits(nc)
    return nc


def _split_multi_waits(nc):
    """Walrus codegen accepts at most one sync-wait per instruction; hoist
    extra waits onto preceding same-engine NoOps (semantically identical —
    the engine stalls at the NoOp instead)."""
    import bass_rust

    for func in nc.m.functions:
        for blk in func.blocks:
            insts = blk.instructions
            new_list = []
            n_split = 0
            for inst in insts:
                si = getattr(inst, "sync_info", None)
                waits = list(si.on_wait) if si is not None else []
                if len(waits) > 1:
                    for k, w in enumerate(waits[:-1]):
                        no = bass_rust.InstNoOp(name=f"{inst.name}-ws{k}")
                        no.engine = inst.engine
                        no.sync_info = bass_rust.SyncInfo(
                            on_wait=[w], on_update=[]
                        )
                        new_list.append(no)
                        n_split += 1
                    inst.sync_info = bass_rust.SyncInfo(
                        on_wait=[waits[-1]], on_update=list(si.on_update)
                    )
                new_list.append(inst)
            if n_split:
                blk.instructions = new_list


@functools.lru_cache(maxsize=8)
def _get_nc(reps=1, parts="full"):
    return _build_nc(reps, parts)


@functools.lru_cache(maxsize=1)
def _get_mask():
    m = np.zeros((128, C, 16, O), dtype=np.float32)
    for p in range(128):
        m[p, :, p // 8, :] = 1
    return np.ascontiguousarray(m.reshape(128, C * 16 * O))


def _squash64(s):
    sq = (s * s).sum(-1, keepdims=True)
    return (sq / (1.0 + sq)) * s / np.sqrt(sq)


def kernel(x, route_weights, capsule_bias):
    global LAST_RESULTS
    from concourse.bass_utils import run_bass_kernel_spmd

    x = np.ascontiguousarray(np.asarray(x, dtype=np.float32))
    W = np.ascontiguousarray(np.asarray(route_weights, dtype=np.float32))
    bias = np.asarray(capsule_bias, dtype=np.float64).reshape(C, O)

    mask = _get_mask()
    in_maps = []
    for k in range(NCORES):
        rs, re = k * RL, (k + 1) * RL
        xs_k = x[:, rs:re, :].reshape(B, RL * I)
        # [C, RL, I, O] -> [RCHUNK, (16r 8i), (c o)]
        ws_k = np.ascontiguousarray(
            W[:, rs:re]
            .reshape(C, RCHUNK, 16, I, O)
            .transpose(1, 2, 3, 0, 4)
            .reshape(RCHUNK, 128, C * O)
        )
        in_maps.append({"xs": np.ascontiguousarray(xs_k), "ws": ws_k, "mk": mask})

    res = run_bass_kernel_spmd(_get_nc(), in_maps, core_ids=list(range(NCORES)))
    LAST_RESULTS = res
    outs = res.results

    s_sum = np.zeros((B, C, O), dtype=np.float64)
    q = np.empty((B, C, R), dtype=np.float32)
    for k in range(NCORES):
        s_sum += np.asarray(outs[k]["s_out"], dtype=np.float64).reshape(B, C, O)
        # q_out: [RCHUNK, B, (c rl)] -> [B, C, RL]
        qk = np.asarray(outs[k]["q_out"]).reshape(RCHUNK, B, C, 16)
        q[:, :, k * RL:(k + 1) * RL] = (
            qk.transpose(1, 2, 0, 3).reshape(B, C, RL)
        )

    # ---- host sparse routing correction (exact, f64) ----
    bs, cs, rs_ = np.nonzero(q < THETA)
    pS = np.einsum(
        "si,sio->so",
        x[bs, rs_].astype(np.float64),
        W[cs, rs_].astype(np.float64),
    )
    qS = (pS * pS).sum(-1)

    L = np.zeros(len(bs), dtype=np.float64)
    out = None
    for it in range(ROUTE_ITERATIONS):
        u = np.expm1(L)
        usum = np.zeros((B, C))
        np.add.at(usum, (bs, cs), u)
        corr = np.zeros((B, C, O))
        np.add.at(corr, (bs, cs), u[:, None] * pS)
        s = (s_sum + corr) / (R + usum)[..., None]
        out = _squash64(s) + bias[None]
        if it < ROUTE_ITERATIONS - 1:
            outS = out[bs, cs]
            d2 = qS - 2.0 * (pS * outS).sum(-1) + (outS * outS).sum(-1)
            L = L + np.exp(-d2 / (SIGMA * SIGMA))

    return out.astype(np.float32)

